# revision 36
# baseline (speedup 1.0000x reference)
"""AFGNet fused attention kernel for 8 TRN2 NeuronCores.

Sharding: core = 2*b + half  (b in 0..3 batches, half in 0..1 query-node halves).
Each core computes out[b, half*1024:(half+1)*1024, :] independently -> no collectives.

Math (per core):
  Z' = [X @ Wqc + bqc | 1]                  (rank-25 bottleneck, ones col folds biases)
  S_h = Z' M_h Z'^T + bf1_h bf2_h           (M_h = Wq'_h Wk'_h^T / sqrt(hd), host-folded)
  S computed TRANSPOSED (keys on partitions) as KF_h^T @ QF_h, K=33
  attn_u = exp(S_T) * exp(fr*learnedT + (1-fr)*fixedT)   (exp-factor trick)
  ctx^T|denom = [v_h|1]^T @ attn_u          (ones row gives softmax denominator)
  out = LN((ctx/denom) @ (Woc Woe) + (boc Woe + boe) + X)
"""

import os
import numpy as np

F_DIM = 64
N_NODES = 2048
N_HEADS = 4
HEAD_DIM = 16
C_DIM = 8
BATCH = 4
NQ = N_NODES // 2  # local queries per core
LN_EPS = 1e-5

LAST_EXEC_NS = None
LAST_RESULTS = None


def _build_bass(fr):
    import concourse.mybir as mybir
    from concourse import bacc
    from concourse import tile
    from concourse import masks
    from contextlib import ExitStack

    f32 = mybir.dt.float32
    bf16 = mybir.dt.bfloat16
    AF = mybir.ActivationFunctionType
    ALU = mybir.AluOpType

    nc = bacc.Bacc()

    # ---- DRAM I/O ----
    xT = nc.dram_tensor("xT", [F_DIM, N_NODES], f32, kind="ExternalInput")
    xTq = nc.dram_tensor("xTq", [F_DIM, NQ], f32, kind="ExternalInput")
    lt = nc.dram_tensor("lt", [N_NODES, NQ], f32, kind="ExternalInput")
    ft = nc.dram_tensor("ft", [N_NODES, NQ], f32, kind="ExternalInput")
    wqc = nc.dram_tensor("wqc", [F_DIM, 24], bf16, kind="ExternalInput")
    bqc = nc.dram_tensor("bqc", [24, 1], f32, kind="ExternalInput")
    mT = nc.dram_tensor("mT", [N_HEADS, 25, 25], bf16, kind="ExternalInput")
    wv = nc.dram_tensor("wv", [25, F_DIM], bf16, kind="ExternalInput")
    bf1tq = nc.dram_tensor("bf1tq", [N_HEADS, C_DIM, NQ], bf16, kind="ExternalInput")
    bf2 = nc.dram_tensor("bf2", [N_HEADS, C_DIM, N_NODES], bf16, kind="ExternalInput")
    wo = nc.dram_tensor("wo", [128, F_DIM], bf16, kind="ExternalInput")
    bo = nc.dram_tensor("bo", [F_DIM, 1], f32, kind="ExternalInput")
    lngb = nc.dram_tensor("lngb", [2, F_DIM], f32, kind="ExternalInput")
    fid = nc.dram_tensor("fid", [128, 128], bf16, kind="ExternalInput")
    out_d = nc.dram_tensor("out", [NQ, F_DIM], f32, kind="ExternalOutput")
    warm_d = nc.dram_tensor("warm", [128, 1], f32, kind="ExternalOutput")

    NMT = N_NODES // 128  # 16 key tiles

    with tile.TileContext(nc) as tc, ExitStack() as ctx:
        const = ctx.enter_context(tc.tile_pool(name="const", bufs=1))

        ident = const.tile([128, 128], f32)
        masks.make_identity(nc, ident[:])
        ones = const.tile([1, 128], f32)
        nc.vector.memset(ones[:], 1.0)
        eps_sb = const.tile([128, 1], f32)
        nc.vector.memset(eps_sb[:], LN_EPS)

        # ---- load small constants ----
        wqc_sb = const.tile([F_DIM, 24], bf16)
        nc.sync.dma_start(wqc_sb[:], wqc[:])
        bqc_sb = const.tile([24, 1], f32)
        nc.sync.dma_start(bqc_sb[:], bqc[:])
        mT_sb = const.tile([25, 100], bf16)
        for h in range(N_HEADS):
            nc.sync.dma_start(mT_sb[:, 25 * h : 25 * h + 25], mT[h])
        wv_sb = const.tile([25, F_DIM], bf16)
        nc.sync.dma_start(wv_sb[:], wv[:])
        wo_sb = const.tile([128, F_DIM], bf16)
        nc.sync.dma_start(wo_sb[:], wo[:])
        bo_sb = const.tile([F_DIM, 1], f32)
        nc.sync.dma_start(bo_sb[:], bo[:])
        lng_sb = const.tile([1, F_DIM], f32)
        nc.sync.dma_start(lng_sb[:], lngb[0:1, :])
        lnb_sb = const.tile([1, F_DIM], f32)
        nc.sync.dma_start(lnb_sb[:], lngb[1:2, :])

        fid_sb = const.tile([128, 128], bf16)
        nc.sync.dma_start(fid_sb[:], fid[:])
        xT_sb = const.tile([F_DIM, N_NODES], f32)
        nc.sync.dma_start(xT_sb[:], xT[:])
        xTq_sb = const.tile([F_DIM, NQ], f32)
        nc.sync.dma_start(xTq_sb[:], xTq[:])
        xTb = const.tile([F_DIM, N_NODES], bf16)
        nc.vector.tensor_copy(xTb[:], xT_sb[:])
        xTqb = const.tile([F_DIM, NQ], bf16)
        nc.vector.tensor_copy(xTqb[:], xTq_sb[:])

        # replicate ln_g, ln_b to [128, 64] via K=1 matmul
        pro_ctx = ExitStack()
        pro = pro_ctx.enter_context(tc.tile_pool(name="pro", bufs=2, space="PSUM"))

        # PE warm-up: dense dependency-free matmul burst to engage the HAM clock
        wur = const.tile([128, 1], f32)
        nc.vector.memset(wur[:], 0.0)
        nc.sync.dma_start(warm_d[:], wur[:])
        g_rep = const.tile([128, F_DIM], f32)
        b_rep = const.tile([128, F_DIM], f32)
        for src_t, dst in ((lng_sb, g_rep), (lnb_sb, b_rep)):
            rp = pro.tile([128, 512], f32, tag="pro")
            nc.tensor.matmul(rp[:, 0:F_DIM], ones[:], src_t[:], start=True, stop=True)
            nc.vector.tensor_copy(dst[:], rp[:, 0:F_DIM])

        # ---- Z'^T for all nodes [25, 2048] and for queries [25, 1024] ----
        zT = const.tile([25, N_NODES], bf16)
        nc.vector.memset(zT[:], 1.0)
        for ck in range(N_NODES // 512):
            zp = pro.tile([128, 512], f32, tag="pro")
            nc.tensor.matmul(zp[0:24, :], wqc_sb[:], xTb[:, 512 * ck : 512 * (ck + 1)], start=True, stop=True)
            nc.vector.tensor_scalar(zT[0:24, 512 * ck : 512 * (ck + 1)], zp[0:24, :], bqc_sb[:], None, ALU.add)
        zTq = const.tile([25, NQ], bf16)
        nc.vector.memset(zTq[:], 1.0)
        for ck in range(NQ // 512):
            zp = pro.tile([128, 512], f32, tag="pro")
            nc.tensor.matmul(zp[0:24, :], wqc_sb[:], xTqb[:, 512 * ck : 512 * (ck + 1)], start=True, stop=True)
            nc.vector.tensor_scalar(zTq[0:24, 512 * ck : 512 * (ck + 1)], zp[0:24, :], bqc_sb[:], None, ALU.add)

        # ---- KF_h [33, 2048] = [M_h Z'^T ; bf2_h],  QF_h [33, 1024] = [Z'^T_q ; bf1tq_h] ----
        kf = []
        qf = []
        for h in range(N_HEADS):
            kfh = const.tile([40, N_NODES], bf16, tag=f"kf{h}")
            nc.vector.memset(kfh[0:32, :], 0.0)
            nc.sync.dma_start(kfh[32:40, :], bf2[h])
            nc.vector.tensor_copy(kfh[0:25, :], zT[:])
            kf.append(kfh)
            qfh = const.tile([40, NQ], bf16, tag=f"qf{h}")
            nc.vector.memset(qfh[0:32, :], 0.0)
            nc.sync.dma_start(qfh[32:40, :], bf1tq[h])
            for ck in range(NQ // 512):
                kp = pro.tile([128, 512], f32, tag="pro")
                nc.tensor.matmul(kp[0:25, :], mT_sb[:, 25 * h : 25 * h + 25], zTq[:, 512 * ck : 512 * (ck + 1)], start=True, stop=True)
                nc.vector.tensor_copy(qfh[0:25, 512 * ck : 512 * (ck + 1)], kp[0:25, :])
            qf.append(qfh)

        # ---- V with ones col per head: [128, 68] per key tile ----
        v_all = const.tile([128, 68 * NMT], bf16)
        nc.vector.memset(v_all[:], 1.0)
        for mt in range(NMT):
            vp = pro.tile([128, 512], f32, tag="pro")
            nc.tensor.matmul(vp[:, 0:F_DIM], zT[:, 128 * mt : 128 * (mt + 1)], wv_sb[:], start=True, stop=True)
            for h in range(N_HEADS):
                nc.vector.tensor_copy(
                    v_all[:, 68 * mt + 17 * h + 1 : 68 * mt + 17 * h + 17],
                    vp[:, 16 * h : 16 * h + 16],
                )

        pro_ctx.close()

        # ---- context accumulators: 2 n-chunks, head h at partitions 32h..32h+16 ----
        cpsum = ctx.enter_context(tc.tile_pool(name="cpsum", bufs=1, space="PSUM"))
        ctxps = [cpsum.tile([128, 512], f32, name=f"ctxacc{nz}", tag=f"ctx{nz}") for nz in range(2)]
        for nz in range(2):
            nc.vector.memset(ctxps[nz][:], 0.0)

        spsum = ctx.enter_context(tc.tile_pool(name="spsum", bufs=3, space="PSUM"))
        gpool = ctx.enter_context(tc.tile_pool(name="gpool", bufs=4))
        apool = ctx.enter_context(tc.tile_pool(name="apool", bufs=3))

        # ---- main loop over 16 key tiles ----
        pend = None
        for mt in range(NMT):
            lt_t = gpool.tile([128, NQ], f32, tag="lt")
            nc.sync.dma_start(lt_t[:], lt[128 * mt : 128 * (mt + 1), :])
            ft_t = gpool.tile([128, NQ], f32, tag="ft")
            nc.sync.dma_start(ft_t[:], ft[128 * mt : 128 * (mt + 1), :])
            # cpre = lt + c*ft ; exp arg = fr*cpre  (c = (1-fr)/fr), assumes fr >= 0.5 handled on host
            t0 = gpool.tile([128, NQ], f32, tag="t0")
            nc.vector.tensor_scalar(t0[:], ft_t[:], float((1.0 - fr) / fr), None, ALU.mult)
            cpre = gpool.tile([128, NQ], f32, tag="cpre")
            nc.vector.tensor_tensor(cpre[:], t0[:], lt_t[:], ALU.add)
            expct = gpool.tile([128, NQ], bf16, tag="expct")
            nc.scalar.activation(expct[:], cpre[:], AF.Exp, scale=float(fr))

            at_list = []
            for h in range(N_HEADS):
                sps = spsum.tile([128, NQ], f32, tag="sps")
                for nz in range(2):
                    nc.tensor.matmul(
                        sps[:, 512 * nz : 512 * (nz + 1)],
                        kf[h][:, 128 * mt : 128 * (mt + 1)],
                        qf[h][:, 512 * nz : 512 * (nz + 1)],
                        start=True,
                        stop=True,
                    )
                exps = apool.tile([128, NQ], bf16, tag="exps")
                nc.scalar.activation(exps[:], sps[:], AF.Exp)
                attnu = apool.tile([128, NQ], bf16, tag=f"attnu{h}")
                nc.vector.tensor_tensor(attnu[:], exps[:], expct[:], ALU.mult)
                at_list.append(attnu)
            if pend is not None:
                _mtP, _atP = pend
                for nz in range(2):
                    for h in range(N_HEADS):
                        nc.tensor.matmul(
                            ctxps[nz][32 * h : 32 * h + 17, :],
                            v_all[:, 68 * _mtP + 17 * h : 68 * _mtP + 17 * h + 17],
                            _atP[h][:, 512 * nz : 512 * (nz + 1)],
                            start=False,
                            stop=False,
                            skip_group_check=True,
                            tile_position=(0, 32 * h),
                        )
            pend = (mt, at_list)

        _mtP, _atP = pend
        for nz in range(2):
            for h in range(N_HEADS):
                nc.tensor.matmul(
                    ctxps[nz][32 * h : 32 * h + 17, :],
                    v_all[:, 68 * _mtP + 17 * h : 68 * _mtP + 17 * h + 17],
                    _atP[h][:, 512 * nz : 512 * (nz + 1)],
                    start=False,
                    stop=False,
                    skip_group_check=True,
                    tile_position=(0, 32 * h),
                )

        # ---- epilogue ----
        epi = ctx.enter_context(tc.tile_pool(name="epi", bufs=2))
        episum = spsum
        for nz in range(2):
            ctxsb = epi.tile([128, 512], bf16, tag="ctxsb")
            nc.vector.memset(ctxsb[:], 0.0)
            for h in range(N_HEADS):
                cu = epi.tile([17, 512], f32, tag="cu")
                nc.vector.tensor_copy(cu[:], ctxps[nz][32 * h : 32 * h + 17, :])
                rd = epi.tile([1, 512], f32, tag="rd")
                nc.vector.reciprocal_approx_fast(rd[:], cu[0:1, :])
                rep = episum.tile([128, NQ], f32, tag="sps", name="rep")
                nc.tensor.matmul(rep[0:17, 0:512], ones[:, 0:17], rd[:], start=True, stop=True)
                nc.vector.tensor_tensor(ctxsb[32 * h : 32 * h + 17, :], cu[0:17, :], rep[0:17, 0:512], ALU.mult)
            pp = episum.tile([128, NQ], f32, tag="sps", name="pp")
            nc.tensor.matmul(pp[0:F_DIM, 0:512], wo_sb[:], ctxsb[:], start=True, stop=True)
            pre1 = epi.tile([F_DIM, 512], f32, tag="pre1")
            nc.vector.tensor_scalar(pre1[:], pp[0:F_DIM, 0:512], bo_sb[:], None, ALU.add)
            pre2 = epi.tile([F_DIM, 512], f32, tag="pre2")
            nc.vector.tensor_tensor(pre2[:], pre1[:], xTq_sb[:, 512 * nz : 512 * (nz + 1)], ALU.add)
            for ck in range(4):
                tp = episum.tile([128, NQ], f32, tag="sps", name="tp")
                nc.tensor.transpose(tp[:, 0:F_DIM], pre2[:, 128 * ck : 128 * (ck + 1)], ident[0:F_DIM, 0:F_DIM])
                # LayerNorm over free dim (64)
                mu = epi.tile([128, 1], f32, tag="mu")
                nc.vector.tensor_reduce(mu[:], tp[:, 0:F_DIM], mybir.AxisListType.X, ALU.add)
                mus = epi.tile([128, 1], f32, tag="mus")
                nc.vector.tensor_scalar(mus[:], mu[:], 1.0 / F_DIM, None, ALU.mult)
                xc = epi.tile([128, F_DIM], f32, tag="xc")
                nc.vector.tensor_scalar(xc[:], tp[:, 0:F_DIM], mus[:], None, ALU.subtract)
                sq = epi.tile([128, F_DIM], f32, tag="sq")
                nc.vector.tensor_tensor(sq[:], xc[:], xc[:], ALU.mult)
                vs = epi.tile([128, 1], f32, tag="vs")
                nc.vector.tensor_reduce(vs[:], sq[:], mybir.AxisListType.X, ALU.add)
                sd = epi.tile([128, 1], f32, tag="sd")
                nc.scalar.activation(sd[:], vs[:], AF.Sqrt, bias=eps_sb[:], scale=float(1.0 / F_DIM))
                rstd = epi.tile([128, 1], f32, tag="rstd")
                nc.vector.reciprocal(rstd[:], sd[:])
                y1 = epi.tile([128, F_DIM], f32, tag="y1")
                nc.vector.tensor_scalar(y1[:], xc[:], rstd[:], None, ALU.mult)
                y2 = epi.tile([128, F_DIM], f32, tag="y2")
                nc.vector.tensor_tensor(y2[:], y1[:], g_rep[:], ALU.mult)
                y3 = epi.tile([128, F_DIM], f32, tag="y3")
                nc.vector.tensor_tensor(y3[:], y2[:], b_rep[:], ALU.add)
                nc.sync.dma_start(out_d[512 * nz + 128 * ck : 512 * nz + 128 * (ck + 1), :], y3[:])

    nc.compile()
    return nc


def _install_ntff_hook():
    """Register the axon NTFF profiling hook that trn_boot skips when
    antenv.axon_hooks is missing, and stub out the artifact upload."""
    import sys
    import types
    try:
        from antenv.axon_hooks import get_axon_ntff_profile_hook  # noqa: F401
        return True
    except ImportError:
        pass
    try:
        from trn_agent_boot.trn_boot import _ntff_profile_via_ctypes
        hook = _ntff_profile_via_ctypes("/opt/axon/libaxon_pjrt.so")
        if hook is None:
            return False
        mod = types.ModuleType("antenv.axon_hooks")
        state = {"hook": hook}
        mod.set_axon_ntff_profile_hook = lambda h: state.__setitem__("hook", h)
        mod.get_axon_ntff_profile_hook = lambda: state["hook"]
        sys.modules["antenv.axon_hooks"] = mod
        import antenv
        antenv.axon_hooks = mod
        import concourse.bass_utils as _bu
        _bu.upload_artifacts = lambda tmpdir: str(tmpdir)
        return True
    except Exception:
        return False


def kernel(features, fixed_graph, learned_graph, Wqc, bqc, Wqe, bqe,
           Woc, boc, Woe, boe, bf1, bf2, graph_fusion, ln_g, ln_b):
    import ml_dtypes
    from concourse.bass_utils import run_bass_kernel_spmd

    global LAST_EXEC_NS, LAST_RESULTS

    f32 = np.float32
    bft = ml_dtypes.bfloat16
    features = np.asarray(features, f32)
    fixed_graph = np.asarray(fixed_graph, f32)
    learned_graph = np.asarray(learned_graph, f32)
    Wqe = np.asarray(Wqe, f32)
    bqe = np.asarray(bqe, f32)

    fr = float(1.0 / (1.0 + np.exp(-float(np.asarray(graph_fusion).reshape(-1)[0]))))
    # kernel folds blend as exp(fr*(lt + c*ft)); keep fr the larger coeff for stability
    swap = fr < 0.5
    if swap:
        fr_eff = 1.0 - fr
        g_a, g_b = fixed_graph, learned_graph  # a gets coeff fr_eff via scale
    else:
        fr_eff = fr
        g_a, g_b = learned_graph, fixed_graph

    Wq = np.vstack([Wqe[:, 0:64], bqe[None, 0:64]])        # [25, 64]
    Wk = np.vstack([Wqe[:, 64:128], bqe[None, 64:128]])
    Wv = np.vstack([Wqe[:, 128:192], bqe[None, 128:192]])
    mT_h = np.stack([
        (Wq[:, 16 * h : 16 * h + 16] @ Wk[:, 16 * h : 16 * h + 16].T) / np.sqrt(HEAD_DIM)
        for h in range(N_HEADS)
    ]).astype(bft)                                          # M_h (lhsT for QF-side fold)
    wo_sq = np.asarray(Woc, f32) @ np.asarray(Woe, f32)        # [64, 64]
    wo_f = np.zeros((128, F_DIM), f32)
    for h in range(N_HEADS):
        wo_f[32 * h + 1 : 32 * h + 17] = wo_sq[16 * h : 16 * h + 16]
    wo_f = wo_f.astype(bft)
    bo_f = (np.asarray(boc, f32) @ np.asarray(Woe, f32) + np.asarray(boe, f32)).astype(f32)

    nc = _build_bass(fr_eff)

    bf1_t = np.ascontiguousarray(np.asarray(bf1, f32).transpose(0, 2, 1)).astype(bft)  # [H, C, N]
    bf2_b = np.asarray(bf2, f32).astype(bft)
    wqc_b = np.asarray(Wqc, f32).astype(bft)
    bqc_c = np.ascontiguousarray(np.asarray(bqc, f32).reshape(24, 1))
    wv_b = Wv.astype(bft)
    lngb = np.stack([np.asarray(ln_g, f32), np.asarray(ln_b, f32)])

    fid_np = (np.eye(128, dtype=f32) * np.float32(fr_eff)).astype(bft)
    in_maps = []
    for core in range(8):
        b, half = core // 2, core % 2
        q0, q1 = half * NQ, (half + 1) * NQ
        ga = g_a if g_a.ndim == 2 else g_a[b]
        gb = g_b if g_b.ndim == 2 else g_b[b]
        in_maps.append({
            "xT": np.ascontiguousarray(features[b].T),
            "xTq": np.ascontiguousarray(features[b, q0:q1].T),
            "lt": np.ascontiguousarray(ga[q0:q1, :].T),
            "ft": np.ascontiguousarray(gb[q0:q1, :].T),
            "wqc": wqc_b,
            "bqc": bqc_c,
            "mT": mT_h,
            "wv": wv_b,
            "bf1tq": np.ascontiguousarray(bf1_t[:, :, q0:q1]),
            "bf2": bf2_b,
            "wo": wo_f,
            "bo": bo_f.reshape(F_DIM, 1),
            "lngb": lngb,
            "fid": fid_np,
        })

    trace = bool(os.environ.get("KERNEL_TRACE"))
    if trace:
        trace = _install_ntff_hook()
    res = run_bass_kernel_spmd(nc, in_maps, core_ids=list(range(8)), trace=trace)
    LAST_RESULTS = res
    LAST_EXEC_NS = res.exec_time_ns

    out = np.empty((BATCH, N_NODES, F_DIM), f32)
    for core in range(8):
        b, half = core // 2, core % 2
        out[b, half * NQ : (half + 1) * NQ] = np.asarray(res.results[core]["out"], f32)
    reg_loss = np.float32(1e-5 / N_NODES)
    return out, reg_loss


# revision 37
# speedup vs baseline: 1.0399x; 1.0399x over previous
"""AFGNet fused attention kernel for 8 TRN2 NeuronCores.

Sharding: core = 2*b + half  (b in 0..3 batches, half in 0..1 query-node halves).
Each core computes out[b, half*1024:(half+1)*1024, :] independently -> no collectives.

Math (per core):
  Z' = [X @ Wqc + bqc | 1]                  (rank-25 bottleneck, ones col folds biases)
  S_h = Z' M_h Z'^T + bf1_h bf2_h           (M_h = Wq'_h Wk'_h^T / sqrt(hd), host-folded)
  S computed TRANSPOSED (keys on partitions) as KF_h^T @ QF_h, K=33
  attn_u = exp(S_T) * exp(fr*learnedT + (1-fr)*fixedT)   (exp-factor trick)
  ctx^T|denom = [v_h|1]^T @ attn_u          (ones row gives softmax denominator)
  out = LN((ctx/denom) @ (Woc Woe) + (boc Woe + boe) + X)
"""

import os
import numpy as np

F_DIM = 64
N_NODES = 2048
N_HEADS = 4
HEAD_DIM = 16
C_DIM = 8
BATCH = 4
NQ = N_NODES // 2  # local queries per core
LN_EPS = 1e-5

LAST_EXEC_NS = None
LAST_RESULTS = None


def _build_bass(fr):
    import concourse.mybir as mybir
    from concourse import bacc
    from concourse import tile
    from concourse import masks
    from contextlib import ExitStack

    f32 = mybir.dt.float32
    bf16 = mybir.dt.bfloat16
    AF = mybir.ActivationFunctionType
    ALU = mybir.AluOpType

    nc = bacc.Bacc()

    # ---- DRAM I/O ----
    xT = nc.dram_tensor("xT", [F_DIM, N_NODES], f32, kind="ExternalInput")
    xTq = nc.dram_tensor("xTq", [F_DIM, NQ], f32, kind="ExternalInput")
    lt = nc.dram_tensor("lt", [N_NODES, NQ], f32, kind="ExternalInput")
    ft = nc.dram_tensor("ft", [N_NODES, NQ], f32, kind="ExternalInput")
    wqc = nc.dram_tensor("wqc", [F_DIM, 24], bf16, kind="ExternalInput")
    bqc = nc.dram_tensor("bqc", [24, 1], f32, kind="ExternalInput")
    mT = nc.dram_tensor("mT", [N_HEADS, 25, 25], bf16, kind="ExternalInput")
    wv = nc.dram_tensor("wv", [25, F_DIM], bf16, kind="ExternalInput")
    bf1tq = nc.dram_tensor("bf1tq", [N_HEADS, C_DIM, NQ], bf16, kind="ExternalInput")
    bf2 = nc.dram_tensor("bf2", [N_HEADS, C_DIM, N_NODES], bf16, kind="ExternalInput")
    wo = nc.dram_tensor("wo", [128, F_DIM], bf16, kind="ExternalInput")
    bo = nc.dram_tensor("bo", [F_DIM, 1], f32, kind="ExternalInput")
    lngb = nc.dram_tensor("lngb", [2, F_DIM], f32, kind="ExternalInput")
    fid = nc.dram_tensor("fid", [128, 128], bf16, kind="ExternalInput")
    out_d = nc.dram_tensor("out", [NQ, F_DIM], f32, kind="ExternalOutput")
    warm_d = nc.dram_tensor("warm", [128, 1], f32, kind="ExternalOutput")

    NMT = N_NODES // 128  # 16 key tiles

    with tile.TileContext(nc) as tc, ExitStack() as ctx:
        const = ctx.enter_context(tc.tile_pool(name="const", bufs=1))

        ident = const.tile([128, 128], f32)
        masks.make_identity(nc, ident[:])
        ones = const.tile([1, 128], f32)
        nc.vector.memset(ones[:], 1.0)
        eps_sb = const.tile([128, 1], f32)
        nc.vector.memset(eps_sb[:], LN_EPS)

        # ---- load small constants ----
        wqc_sb = const.tile([F_DIM, 24], bf16)
        nc.sync.dma_start(wqc_sb[:], wqc[:])
        bqc_sb = const.tile([24, 1], f32)
        nc.sync.dma_start(bqc_sb[:], bqc[:])
        mT_sb = const.tile([25, 100], bf16)
        for h in range(N_HEADS):
            nc.sync.dma_start(mT_sb[:, 25 * h : 25 * h + 25], mT[h])
        wv_sb = const.tile([25, F_DIM], bf16)
        nc.sync.dma_start(wv_sb[:], wv[:])
        wo_sb = const.tile([128, F_DIM], bf16)
        nc.sync.dma_start(wo_sb[:], wo[:])
        bo_sb = const.tile([F_DIM, 1], f32)
        nc.sync.dma_start(bo_sb[:], bo[:])
        lng_sb = const.tile([1, F_DIM], f32)
        nc.sync.dma_start(lng_sb[:], lngb[0:1, :])
        lnb_sb = const.tile([1, F_DIM], f32)
        nc.sync.dma_start(lnb_sb[:], lngb[1:2, :])

        fid_sb = const.tile([128, 128], bf16)
        nc.sync.dma_start(fid_sb[:], fid[:])
        xT_sb = const.tile([F_DIM, N_NODES], f32)
        nc.sync.dma_start(xT_sb[:], xT[:])
        xTq_sb = const.tile([F_DIM, NQ], f32)
        nc.sync.dma_start(xTq_sb[:], xTq[:])
        xTb = const.tile([F_DIM, N_NODES], bf16)
        nc.vector.tensor_copy(xTb[:], xT_sb[:])
        xTqb = const.tile([F_DIM, NQ], bf16)
        nc.vector.tensor_copy(xTqb[:], xTq_sb[:])

        # replicate ln_g, ln_b to [128, 64] via K=1 matmul
        pro_ctx = ExitStack()
        pro = pro_ctx.enter_context(tc.tile_pool(name="pro", bufs=2, space="PSUM"))

        # PE warm-up: dense dependency-free matmul burst to engage the HAM clock
        wur = const.tile([128, 1], f32)
        nc.vector.memset(wur[:], 0.0)
        nc.sync.dma_start(warm_d[:], wur[:])
        g_rep = const.tile([128, F_DIM], f32)
        b_rep = const.tile([128, F_DIM], f32)
        for src_t, dst in ((lng_sb, g_rep), (lnb_sb, b_rep)):
            rp = pro.tile([128, 512], f32, tag="pro")
            nc.tensor.matmul(rp[:, 0:F_DIM], ones[:], src_t[:], start=True, stop=True)
            nc.vector.tensor_copy(dst[:], rp[:, 0:F_DIM])

        # ---- Z'^T for all nodes [25, 2048] and for queries [25, 1024] ----
        zT = const.tile([25, N_NODES], bf16)
        nc.vector.memset(zT[:], 1.0)
        for ck in range(N_NODES // 512):
            zp = pro.tile([128, 512], f32, tag="pro")
            nc.tensor.matmul(zp[0:24, :], wqc_sb[:], xTb[:, 512 * ck : 512 * (ck + 1)], start=True, stop=True)
            nc.vector.tensor_scalar(zT[0:24, 512 * ck : 512 * (ck + 1)], zp[0:24, :], bqc_sb[:], None, ALU.add)
        zTq = const.tile([25, NQ], bf16)
        nc.vector.memset(zTq[:], 1.0)
        for ck in range(NQ // 512):
            zp = pro.tile([128, 512], f32, tag="pro")
            nc.tensor.matmul(zp[0:24, :], wqc_sb[:], xTqb[:, 512 * ck : 512 * (ck + 1)], start=True, stop=True)
            nc.vector.tensor_scalar(zTq[0:24, 512 * ck : 512 * (ck + 1)], zp[0:24, :], bqc_sb[:], None, ALU.add)

        # ---- KF_h [33, 2048] = [M_h Z'^T ; bf2_h],  QF_h [33, 1024] = [Z'^T_q ; bf1tq_h] ----
        kf = []
        qf = []
        for h in range(N_HEADS):
            kfh = const.tile([40, N_NODES], bf16, tag=f"kf{h}")
            nc.vector.memset(kfh[0:32, :], 0.0)
            nc.sync.dma_start(kfh[32:40, :], bf2[h])
            nc.vector.tensor_copy(kfh[0:25, :], zT[:])
            kf.append(kfh)
            qfh = const.tile([40, NQ], bf16, tag=f"qf{h}")
            nc.vector.memset(qfh[0:32, :], 0.0)
            nc.sync.dma_start(qfh[32:40, :], bf1tq[h])
            for ck in range(NQ // 512):
                kp = pro.tile([128, 512], f32, tag="pro")
                nc.tensor.matmul(kp[0:25, :], mT_sb[:, 25 * h : 25 * h + 25], zTq[:, 512 * ck : 512 * (ck + 1)], start=True, stop=True)
                nc.vector.tensor_copy(qfh[0:25, 512 * ck : 512 * (ck + 1)], kp[0:25, :])
            qf.append(qfh)

        # ---- V with ones col per head: [128, 68] per key tile ----
        v_all = const.tile([128, 68 * NMT], bf16)
        nc.vector.memset(v_all[:], 1.0)
        for mt in range(NMT):
            vp = pro.tile([128, 512], f32, tag="pro")
            nc.tensor.matmul(vp[:, 0:F_DIM], zT[:, 128 * mt : 128 * (mt + 1)], wv_sb[:], start=True, stop=True)
            for h in range(N_HEADS):
                nc.vector.tensor_copy(
                    v_all[:, 68 * mt + 17 * h + 1 : 68 * mt + 17 * h + 17],
                    vp[:, 16 * h : 16 * h + 16],
                )

        pro_ctx.close()

        # ---- context accumulators: 2 n-chunks, head h at partitions 32h..32h+16 ----
        cpsum = ctx.enter_context(tc.tile_pool(name="cpsum", bufs=1, space="PSUM"))
        ctxps = [cpsum.tile([128, 512], f32, name=f"ctxacc{nz}", tag=f"ctx{nz}") for nz in range(2)]
        for nz in range(2):
            nc.vector.memset(ctxps[nz][:], 0.0)

        spsum = ctx.enter_context(tc.tile_pool(name="spsum", bufs=3, space="PSUM"))
        gpool = ctx.enter_context(tc.tile_pool(name="gpool", bufs=4))
        apool = ctx.enter_context(tc.tile_pool(name="apool", bufs=3))

        # ---- main loop over 16 key tiles ----
        pend = None
        for mt in range(NMT):
            lt_t = gpool.tile([128, NQ], f32, tag="lt")
            nc.sync.dma_start(lt_t[:], lt[128 * mt : 128 * (mt + 1), :])
            ft_t = gpool.tile([128, NQ], f32, tag="ft")
            nc.sync.dma_start(ft_t[:], ft[128 * mt : 128 * (mt + 1), :])
            # cpre = lt + c*ft ; exp arg = fr*cpre  (c = (1-fr)/fr), assumes fr >= 0.5 handled on host
            t0 = gpool.tile([128, NQ], f32, tag="t0")
            nc.vector.tensor_scalar(t0[:], ft_t[:], float((1.0 - fr) / fr), None, ALU.mult)
            cpre = gpool.tile([128, NQ], f32, tag="cpre")
            nc.vector.tensor_tensor(cpre[:], t0[:], lt_t[:], ALU.add)
            expct = gpool.tile([128, NQ], bf16, tag="expct")
            nc.scalar.activation(expct[:], cpre[:], AF.Exp, scale=float(fr))

            at_list = []
            for h in range(N_HEADS):
                sps = spsum.tile([128, NQ], f32, tag="sps")
                for nz in range(2):
                    nc.tensor.matmul(
                        sps[:, 512 * nz : 512 * (nz + 1)],
                        kf[h][:, 128 * mt : 128 * (mt + 1)],
                        qf[h][:, 512 * nz : 512 * (nz + 1)],
                        start=True,
                        stop=True,
                    )
                exps = apool.tile([128, NQ], bf16, tag="exps")
                attnu = apool.tile([128, NQ], bf16, tag=f"attnu{h}")
                for nz in range(2):
                    sl = slice(512 * nz, 512 * (nz + 1))
                    nc.scalar.activation(exps[:, sl], sps[:, sl], AF.Exp)
                    nc.vector.tensor_tensor(attnu[:, sl], exps[:, sl], expct[:, sl], ALU.mult)
                at_list.append(attnu)
            if pend is not None:
                _mtP, _atP = pend
                for nz in range(2):
                    for h in range(N_HEADS):
                        nc.tensor.matmul(
                            ctxps[nz][32 * h : 32 * h + 17, :],
                            v_all[:, 68 * _mtP + 17 * h : 68 * _mtP + 17 * h + 17],
                            _atP[h][:, 512 * nz : 512 * (nz + 1)],
                            start=False,
                            stop=False,
                            skip_group_check=True,
                            tile_position=(0, 32 * h),
                        )
            pend = (mt, at_list)

        _mtP, _atP = pend
        for nz in range(2):
            for h in range(N_HEADS):
                nc.tensor.matmul(
                    ctxps[nz][32 * h : 32 * h + 17, :],
                    v_all[:, 68 * _mtP + 17 * h : 68 * _mtP + 17 * h + 17],
                    _atP[h][:, 512 * nz : 512 * (nz + 1)],
                    start=False,
                    stop=False,
                    skip_group_check=True,
                    tile_position=(0, 32 * h),
                )

        # ---- epilogue ----
        epi = ctx.enter_context(tc.tile_pool(name="epi", bufs=2))
        episum = spsum
        for nz in range(2):
            ctxsb = epi.tile([128, 512], bf16, tag="ctxsb")
            nc.vector.memset(ctxsb[:], 0.0)
            for h in range(N_HEADS):
                cu = epi.tile([17, 512], f32, tag="cu")
                nc.vector.tensor_copy(cu[:], ctxps[nz][32 * h : 32 * h + 17, :])
                rd = epi.tile([1, 512], f32, tag="rd")
                nc.vector.reciprocal_approx_fast(rd[:], cu[0:1, :])
                rep = episum.tile([128, NQ], f32, tag="sps", name="rep")
                nc.tensor.matmul(rep[0:17, 0:512], ones[:, 0:17], rd[:], start=True, stop=True)
                nc.vector.tensor_tensor(ctxsb[32 * h : 32 * h + 17, :], cu[0:17, :], rep[0:17, 0:512], ALU.mult)
            pp = episum.tile([128, NQ], f32, tag="sps", name="pp")
            nc.tensor.matmul(pp[0:F_DIM, 0:512], wo_sb[:], ctxsb[:], start=True, stop=True)
            pre1 = epi.tile([F_DIM, 512], f32, tag="pre1")
            nc.vector.tensor_scalar(pre1[:], pp[0:F_DIM, 0:512], bo_sb[:], None, ALU.add)
            pre2 = epi.tile([F_DIM, 512], f32, tag="pre2")
            nc.vector.tensor_tensor(pre2[:], pre1[:], xTq_sb[:, 512 * nz : 512 * (nz + 1)], ALU.add)
            for ck in range(4):
                tp = episum.tile([128, NQ], f32, tag="sps", name="tp")
                nc.tensor.transpose(tp[:, 0:F_DIM], pre2[:, 128 * ck : 128 * (ck + 1)], ident[0:F_DIM, 0:F_DIM])
                # LayerNorm over free dim (64)
                mu = epi.tile([128, 1], f32, tag="mu")
                nc.vector.tensor_reduce(mu[:], tp[:, 0:F_DIM], mybir.AxisListType.X, ALU.add)
                mus = epi.tile([128, 1], f32, tag="mus")
                nc.vector.tensor_scalar(mus[:], mu[:], 1.0 / F_DIM, None, ALU.mult)
                xc = epi.tile([128, F_DIM], f32, tag="xc")
                nc.vector.tensor_scalar(xc[:], tp[:, 0:F_DIM], mus[:], None, ALU.subtract)
                sq = epi.tile([128, F_DIM], f32, tag="sq")
                nc.vector.tensor_tensor(sq[:], xc[:], xc[:], ALU.mult)
                vs = epi.tile([128, 1], f32, tag="vs")
                nc.vector.tensor_reduce(vs[:], sq[:], mybir.AxisListType.X, ALU.add)
                sd = epi.tile([128, 1], f32, tag="sd")
                nc.scalar.activation(sd[:], vs[:], AF.Sqrt, bias=eps_sb[:], scale=float(1.0 / F_DIM))
                rstd = epi.tile([128, 1], f32, tag="rstd")
                nc.vector.reciprocal(rstd[:], sd[:])
                y1 = epi.tile([128, F_DIM], f32, tag="y1")
                nc.vector.tensor_scalar(y1[:], xc[:], rstd[:], None, ALU.mult)
                y2 = epi.tile([128, F_DIM], f32, tag="y2")
                nc.vector.tensor_tensor(y2[:], y1[:], g_rep[:], ALU.mult)
                y3 = epi.tile([128, F_DIM], f32, tag="y3")
                nc.vector.tensor_tensor(y3[:], y2[:], b_rep[:], ALU.add)
                nc.sync.dma_start(out_d[512 * nz + 128 * ck : 512 * nz + 128 * (ck + 1), :], y3[:])

    nc.compile()
    return nc


def _install_ntff_hook():
    """Register the axon NTFF profiling hook that trn_boot skips when
    antenv.axon_hooks is missing, and stub out the artifact upload."""
    import sys
    import types
    try:
        from antenv.axon_hooks import get_axon_ntff_profile_hook  # noqa: F401
        return True
    except ImportError:
        pass
    try:
        from trn_agent_boot.trn_boot import _ntff_profile_via_ctypes
        hook = _ntff_profile_via_ctypes("/opt/axon/libaxon_pjrt.so")
        if hook is None:
            return False
        mod = types.ModuleType("antenv.axon_hooks")
        state = {"hook": hook}
        mod.set_axon_ntff_profile_hook = lambda h: state.__setitem__("hook", h)
        mod.get_axon_ntff_profile_hook = lambda: state["hook"]
        sys.modules["antenv.axon_hooks"] = mod
        import antenv
        antenv.axon_hooks = mod
        import concourse.bass_utils as _bu
        _bu.upload_artifacts = lambda tmpdir: str(tmpdir)
        return True
    except Exception:
        return False


def kernel(features, fixed_graph, learned_graph, Wqc, bqc, Wqe, bqe,
           Woc, boc, Woe, boe, bf1, bf2, graph_fusion, ln_g, ln_b):
    import ml_dtypes
    from concourse.bass_utils import run_bass_kernel_spmd

    global LAST_EXEC_NS, LAST_RESULTS

    f32 = np.float32
    bft = ml_dtypes.bfloat16
    features = np.asarray(features, f32)
    fixed_graph = np.asarray(fixed_graph, f32)
    learned_graph = np.asarray(learned_graph, f32)
    Wqe = np.asarray(Wqe, f32)
    bqe = np.asarray(bqe, f32)

    fr = float(1.0 / (1.0 + np.exp(-float(np.asarray(graph_fusion).reshape(-1)[0]))))
    # kernel folds blend as exp(fr*(lt + c*ft)); keep fr the larger coeff for stability
    swap = fr < 0.5
    if swap:
        fr_eff = 1.0 - fr
        g_a, g_b = fixed_graph, learned_graph  # a gets coeff fr_eff via scale
    else:
        fr_eff = fr
        g_a, g_b = learned_graph, fixed_graph

    Wq = np.vstack([Wqe[:, 0:64], bqe[None, 0:64]])        # [25, 64]
    Wk = np.vstack([Wqe[:, 64:128], bqe[None, 64:128]])
    Wv = np.vstack([Wqe[:, 128:192], bqe[None, 128:192]])
    mT_h = np.stack([
        (Wq[:, 16 * h : 16 * h + 16] @ Wk[:, 16 * h : 16 * h + 16].T) / np.sqrt(HEAD_DIM)
        for h in range(N_HEADS)
    ]).astype(bft)                                          # M_h (lhsT for QF-side fold)
    wo_sq = np.asarray(Woc, f32) @ np.asarray(Woe, f32)        # [64, 64]
    wo_f = np.zeros((128, F_DIM), f32)
    for h in range(N_HEADS):
        wo_f[32 * h + 1 : 32 * h + 17] = wo_sq[16 * h : 16 * h + 16]
    wo_f = wo_f.astype(bft)
    bo_f = (np.asarray(boc, f32) @ np.asarray(Woe, f32) + np.asarray(boe, f32)).astype(f32)

    nc = _build_bass(fr_eff)

    bf1_t = np.ascontiguousarray(np.asarray(bf1, f32).transpose(0, 2, 1)).astype(bft)  # [H, C, N]
    bf2_b = np.asarray(bf2, f32).astype(bft)
    wqc_b = np.asarray(Wqc, f32).astype(bft)
    bqc_c = np.ascontiguousarray(np.asarray(bqc, f32).reshape(24, 1))
    wv_b = Wv.astype(bft)
    lngb = np.stack([np.asarray(ln_g, f32), np.asarray(ln_b, f32)])

    fid_np = (np.eye(128, dtype=f32) * np.float32(fr_eff)).astype(bft)
    in_maps = []
    for core in range(8):
        b, half = core // 2, core % 2
        q0, q1 = half * NQ, (half + 1) * NQ
        ga = g_a if g_a.ndim == 2 else g_a[b]
        gb = g_b if g_b.ndim == 2 else g_b[b]
        in_maps.append({
            "xT": np.ascontiguousarray(features[b].T),
            "xTq": np.ascontiguousarray(features[b, q0:q1].T),
            "lt": np.ascontiguousarray(ga[q0:q1, :].T),
            "ft": np.ascontiguousarray(gb[q0:q1, :].T),
            "wqc": wqc_b,
            "bqc": bqc_c,
            "mT": mT_h,
            "wv": wv_b,
            "bf1tq": np.ascontiguousarray(bf1_t[:, :, q0:q1]),
            "bf2": bf2_b,
            "wo": wo_f,
            "bo": bo_f.reshape(F_DIM, 1),
            "lngb": lngb,
            "fid": fid_np,
        })

    trace = bool(os.environ.get("KERNEL_TRACE"))
    if trace:
        trace = _install_ntff_hook()
    res = run_bass_kernel_spmd(nc, in_maps, core_ids=list(range(8)), trace=trace)
    LAST_RESULTS = res
    LAST_EXEC_NS = res.exec_time_ns

    out = np.empty((BATCH, N_NODES, F_DIM), f32)
    for core in range(8):
        b, half = core // 2, core % 2
        out[b, half * NQ : (half + 1) * NQ] = np.asarray(res.results[core]["out"], f32)
    reg_loss = np.float32(1e-5 / N_NODES)
    return out, reg_loss
